# revision 6
# baseline (speedup 1.0000x reference)
"""AFNO2D Trainium2 kernel — 8 NeuronCores, no collectives.

Sharding: core = (b, c_half): b = core // 2, channels c_half*384 .. +384
(4 independent MLP blocks of 96 channels per core). Every stage is local.

The 2D Hartley transform (Re(FFT2) - Im(FFT2)) and its inverse are dense
matmuls against precomputed cos/sin matrices (H=90, W=180 fixed):
  MM1 (fp8 DoubleRow): P = PW @ x, M = MW @ x  (contract w=180 as [90,2])
  MM2: Xk = RC @ P + RS @ M                    (contract h)
  block-MLP per spectral point (2 layers, relu, softshrink)
  RI:  T = RC @ s, U = RS @ s                  (contract k1)
  WI:  corr = CWI @ [T;U]                      (contract m2-pad=128)

Partition-dim swaps between stages run as single batched DMA-crossbar
transposes (16x128 tiles), keeping the PE free for matmuls.

The residual add (y = x + corr) happens on the host: x is uploaded
pre-quantized to fp8 (only feeds the ~1%-magnitude spectral correction),
the correction is stored as bf16, and the f32 residual never loses
precision.
"""

import sys
import numpy as np

sys.path.insert(0, "/opt/trn_rl_repo")

import ml_dtypes  # noqa: E402
import concourse.bass as bass  # noqa: E402
import concourse.mybir as mybir  # noqa: E402
import concourse.tile as tile  # noqa: E402
from concourse import bacc  # noqa: E402
from concourse.bass_utils import run_bass_kernel_spmd  # noqa: E402

# problem constants (hardcoded per spec)
B, H, W, C = 4, 90, 180, 768
K2 = 46                 # kept width modes
NB, BLK = 8, 96
LAM = 0.01
CPC = 384               # channels per core
NBPC = 4                # MLP blocks per core
NPASS = 4               # c-passes of 96 channels (1 block) per core
HCH = 5                 # h-chunk -> N=480 for MM1/WI
CCH = 10                # c-chunk -> N=460 for MM2
K2CH = 5                # k2-chunk -> N=450 (MLP) / 480 (RI)
YGRP = 3                # WI h-chunks batched per store DMA
MU = 64                 # free-dim offset of the M/U halves (m2 hi part)

F32 = mybir.dt.float32
BF16 = mybir.dt.bfloat16
FP8 = mybir.dt.float8e4
BF = ml_dtypes.bfloat16
E4 = ml_dtypes.float8_e4m3


def _make_host_mats():
    w = np.arange(W)
    h = np.arange(H)
    k2 = np.arange(K2)
    beta = 2 * np.pi * np.outer(k2, w) / W            # [K2, W]
    PW = (np.cos(beta) + np.sin(beta)) / np.sqrt(W)
    MW = (np.cos(beta) - np.sin(beta)) / np.sqrt(W)
    # MM1 DoubleRow lhsT: [P/M, w', j, 48] (col 46,47 zero pad)
    wf8 = np.zeros((2, 90, 2, 48), np.float32)
    for j in range(2):
        wf8[0, :, j, :K2] = PW[:, j * 90:(j + 1) * 90].T
        wf8[1, :, j, :K2] = MW[:, j * 90:(j + 1) * 90].T
    alpha = 2 * np.pi * np.outer(h, h) / H            # [H, H] symmetric
    RC = np.cos(alpha) / np.sqrt(H)
    RS = np.sin(alpha) / np.sqrt(H)
    # WI lhsT [128, j, w']: rows k2 = cos, rows 64+k2 = -sin, rest zero
    cwi = np.zeros((128, 2, 90), np.float32)
    for j in range(2):
        ww = w[j * 90:(j + 1) * 90]
        bb = 2 * np.pi * np.outer(k2, ww) / W
        cwi[:K2, j, :] = np.cos(bb) / np.sqrt(W)
        cwi[MU:MU + K2, j, :] = -np.sin(bb) / np.sqrt(W)
    return (wf8.astype(E4), RC.astype(BF), RS.astype(BF), cwi.astype(BF))


def _build_graph():
    nc = bacc.Bacc("TRN2", target_bir_lowering=False, debug=False, num_devices=8)

    x8_ext = nc.declare_dram_parameter("x8", [90, 2, NPASS, H, BLK], FP8,
                                       isOutput=False)
    y_ext = nc.declare_dram_parameter("y", [NPASS, 2, 90, H, BLK], BF16,
                                      isOutput=True)
    wf8_ext = nc.declare_dram_parameter("wf8", [2, 90, 2, 48], FP8, isOutput=False)
    rc_ext = nc.declare_dram_parameter("rc", [H, H], BF16, isOutput=False)
    rs_ext = nc.declare_dram_parameter("rs", [H, H], BF16, isOutput=False)
    cwi_ext = nc.declare_dram_parameter("cwi", [128, 2, 90], BF16, isOutput=False)
    w1_ext = nc.declare_dram_parameter("w1b", [NBPC, BLK, BLK], BF16, isOutput=False)
    w2_ext = nc.declare_dram_parameter("w2b", [NBPC, BLK, BLK], BF16, isOutput=False)
    b1_ext = nc.declare_dram_parameter("b1b", [BLK, NBPC], F32, isOutput=False)
    # b2 folded with softshrink: b2m = b2 - lam (relu path 1), b2p = -b2 - lam
    b2m_ext = nc.declare_dram_parameter("b2m", [BLK, NBPC], F32, isOutput=False)
    b2p_ext = nc.declare_dram_parameter("b2p", [BLK, NBPC], F32, isOutput=False)

    DR = mybir.MatmulPerfMode.DoubleRow

    with tile.TileContext(nc) as tc:
        with (
            tc.tile_pool(name="consts", bufs=1) as consts,
            tc.tile_pool(name="x8", bufs=2) as x8_pool,
            tc.tile_pool(name="mid", bufs=1) as mid,
            tc.tile_pool(name="shr", bufs=2) as shr_pool,
            tc.tile_pool(name="ytile", bufs=3) as y_pool,
            tc.tile_pool(name="mmps", bufs=4, space="PSUM") as mm_psum,
        ):
            # ---- constants to SBUF ----
            wfP_sb = consts.tile([90, 2, 48], FP8)
            nc.sync.dma_start(wfP_sb[:], wf8_ext[0])
            wfM_sb = consts.tile([90, 2, 48], FP8)
            nc.sync.dma_start(wfM_sb[:], wf8_ext[1])
            rc_sb = consts.tile([H, H], BF16)
            nc.sync.dma_start(rc_sb[:], rc_ext[:])
            rs_sb = consts.tile([H, H], BF16)
            nc.sync.dma_start(rs_sb[:], rs_ext[:])
            cwi_sb = consts.tile([128, 2, 90], BF16)
            nc.sync.dma_start(cwi_sb[:], cwi_ext[:])
            w1_sb = consts.tile([BLK, NBPC, BLK], BF16)
            nc.sync.dma_start(w1_sb[:], w1_ext[:].rearrange("n i o -> i n o"))
            w2_sb = consts.tile([BLK, NBPC, BLK], BF16)
            nc.sync.dma_start(w2_sb[:], w2_ext[:].rearrange("n i o -> i n o"))
            b1_sb = consts.tile([BLK, NBPC], F32)
            nc.sync.dma_start(b1_sb[:], b1_ext[:])
            b2m_sb = consts.tile([BLK, NBPC], F32)
            nc.sync.dma_start(b2m_sb[:], b2m_ext[:])
            b2p_sb = consts.tile([BLK, NBPC], F32)
            nc.sync.dma_start(b2p_sb[:], b2p_ext[:])

            # ---- persistent intermediates (single-buffered) ----
            s1 = mid.tile([112, BLK, 128], BF16)      # [m2lay, c, h]
            s1t = mid.tile([128, BLK, 112], BF16)     # [h, c, m2lay]
            xk = mid.tile([96, K2, 128], BF16)        # [k1, k2, c]
            xkt = mid.tile([128, K2, 96], BF16)       # [c, k2, k1]
            o1 = mid.tile([BLK, K2, H], BF16)         # [c, k2, k1]
            o2s = mid.tile([BLK, K2, 128], BF16)      # [c, k2, k1pad]
            s3 = mid.tile([128, K2, BLK], BF16)       # [k1pad, k2, c]
            ttut = mid.tile([96, BLK, 128], BF16)     # [h, c, m2lay]
            tu = mid.tile([128, BLK, 96], BF16)       # [m2lay, c, h]
            # WI contracts all 128 tu rows against cwi (zero rows at the
            # pads); junk there must be 0.0, not NaN.
            nc.vector.memset(ttut[:, :, K2:MU], 0.0)
            nc.vector.memset(ttut[:, :, MU + K2:128], 0.0)
            # junk h-cols of o2s feed the k1 pad rows of s3 (never read)
            nc.vector.memset(o2s[:, :, H:128], 0.0)

            x8s = []
            for p in range(NPASS):
                t = x8_pool.tile([90, 2, H, BLK], FP8, tag="x8")
                x8s.append(t)
            nc.sync.dma_start(x8s[0][:], x8_ext[:, :, 0, :, :])

            for p in range(NPASS):
                x8t = x8s[p]
                if p + 1 < NPASS:
                    nc.sync.dma_start(x8s[p + 1][:], x8_ext[:, :, p + 1, :, :])

                # ---- MM1 (fp8 DR): contract w, out P/M halves ----
                for hi in range(H // HCH):
                    hs = slice(hi * HCH, (hi + 1) * HCH)
                    rhs = x8t[:, :, hs, :]
                    psP = mm_psum.tile([48, HCH, BLK], F32, tag="mm")
                    nc.tensor.matmul(psP[:], lhsT=wfP_sb[:], rhs=rhs,
                                     start=True, stop=True, perf_mode=DR)
                    psM = mm_psum.tile([48, HCH, BLK], F32, tag="mm")
                    nc.tensor.matmul(psM[:], lhsT=wfM_sb[:], rhs=rhs,
                                     start=True, stop=True, perf_mode=DR)
                    dstP = s1[0:K2, :, hs].rearrange("p c h -> p h c")
                    dstM = s1[MU:MU + K2, :, hs].rearrange("p c h -> p h c")
                    if hi % 2 == 0:
                        nc.scalar.copy(dstP, psP[0:K2, :, :])
                        nc.vector.tensor_copy(dstM, psM[0:K2, :, :])
                    else:
                        nc.vector.tensor_copy(dstP, psP[0:K2, :, :])
                        nc.scalar.copy(dstM, psM[0:K2, :, :])

                # ---- T1 (xbar): [m2lay, c, h] -> [h, c, m2lay] ----
                nc.scalar.dma_start(s1t[:], s1[:].rearrange("p c h -> p (c h)"),
                                    transpose=True)

                # ---- MM2: contract h; Xk = RC@P + RS@M, out [k1, c, k2] ----
                ncch = (BLK + CCH - 1) // CCH
                for ci in range(ncch):
                    c0 = ci * CCH
                    cn = min(CCH, BLK - c0)
                    cs = bass.ds(c0, cn)
                    ps = mm_psum.tile([H, CCH, K2], F32, tag="mm")
                    nc.tensor.matmul(
                        ps[:, 0:cn, :], lhsT=rc_sb[:],
                        rhs=s1t[0:H, cs, 0:K2],
                        start=True, stop=False)
                    nc.tensor.matmul(
                        ps[:, 0:cn, :], lhsT=rs_sb[:],
                        rhs=s1t[0:H, cs, MU:MU + K2],
                        start=False, stop=True)
                    dst = xk[0:H, :, cs].rearrange("p k c -> p c k")
                    if ci % 2 == 0:
                        nc.scalar.copy(dst, ps[:, 0:cn, :])
                    else:
                        nc.vector.tensor_copy(dst, ps[:, 0:cn, :])

                # ---- T2 (xbar): [k1, k2, c] -> [c, k2, k1] ----
                nc.sync.dma_start(xkt[:], xk[:].rearrange("p k c -> p (k c)"),
                                    transpose=True)

                # ---- MLP layer 1 + bias + relu ----
                nk2 = (K2 + K2CH - 1) // K2CH
                for ki in range(nk2):
                    k0 = ki * K2CH
                    kn = min(K2CH, K2 - k0)
                    ks = bass.ds(k0, kn)
                    ps = mm_psum.tile([BLK, K2CH, H], F32, tag="mm")
                    pss = ps[:, 0:kn, :]
                    nc.tensor.matmul(
                        pss, lhsT=w1_sb[:, p, :],
                        rhs=xkt[0:96, ks, 0:H],
                        start=True, stop=True)
                    nc.scalar.activation(
                        o1[:, ks, :],
                        pss, mybir.ActivationFunctionType.Relu,
                        bias=b1_sb[:, p:p + 1])

                # ---- MLP layer 2 + bias + softshrink ----
                for ki in range(nk2):
                    k0 = ki * K2CH
                    kn = min(K2CH, K2 - k0)
                    ks = bass.ds(k0, kn)
                    ps = mm_psum.tile([BLK, K2CH, H], F32, tag="mm")
                    pss = ps[:, 0:kn, :]
                    nc.tensor.matmul(
                        pss, lhsT=w2_sb[:, p, :],
                        rhs=o1[:, ks, :],
                        start=True, stop=True)
                    # softshrink(v + b2) = relu(v + b2 - lam) - relu(-v - b2 - lam)
                    sp = shr_pool.tile([BLK, K2CH, H], BF16, tag="shr_p")
                    sn = shr_pool.tile([BLK, K2CH, H], BF16, tag="shr_n")
                    nc.scalar.activation(
                        sp[:, 0:kn, :], pss, mybir.ActivationFunctionType.Relu,
                        bias=b2m_sb[:, p:p + 1], scale=1.0)
                    nc.scalar.activation(
                        sn[:, 0:kn, :], pss, mybir.ActivationFunctionType.Relu,
                        bias=b2p_sb[:, p:p + 1], scale=-1.0)
                    nc.vector.tensor_sub(
                        o2s[:, ks, 0:H],
                        sp[:, 0:kn, :], sn[:, 0:kn, :])

                # ---- T3 (xbar): [c, k2, k1pad] -> [k1pad, k2, c] ----
                nc.scalar.dma_start(s3[:], o2s[:].rearrange("c k i -> c (k i)"),
                                    transpose=True)

                # ---- RI: contract k1; T = RC@s3, U = RS@s3 ----
                for ki in range(nk2):
                    k0 = ki * K2CH
                    kn = min(K2CH, K2 - k0)
                    rhs = s3[0:H, bass.ds(k0, kn), :]
                    psT = mm_psum.tile([H, K2CH, BLK], F32, tag="mm")
                    nc.tensor.matmul(psT[:, 0:kn, :],
                                     lhsT=rc_sb[:], rhs=rhs,
                                     start=True, stop=True)
                    nc.scalar.copy(
                        ttut[0:H, :, bass.ds(k0, kn)].rearrange("h c m -> h m c"),
                        psT[:, 0:kn, :])
                    psU = mm_psum.tile([H, K2CH, BLK], F32, tag="mm")
                    nc.tensor.matmul(psU[:, 0:kn, :],
                                     lhsT=rs_sb[:], rhs=rhs,
                                     start=True, stop=True)
                    nc.vector.tensor_copy(
                        ttut[0:H, :, bass.ds(MU + k0, kn)]
                        .rearrange("h c m -> h m c"),
                        psU[:, 0:kn, :])

                # ---- T4 (xbar): [h, c, m2lay] -> [m2lay, c, h] ----
                nc.sync.dma_start(tu[:], ttut[:].rearrange("h c m -> h (c m)"),
                                    transpose=True)

                # ---- WI: contract m2 (128-pad), store bf16 correction ----
                for j in range(2):
                    for hg in range(H // (HCH * YGRP)):
                        h0g = hg * HCH * YGRP
                        ysb = y_pool.tile([90, HCH * YGRP, BLK], BF16, tag="y")
                        for si in range(YGRP):
                            h0 = h0g + si * HCH
                            hs = slice(h0, h0 + HCH)
                            ps = mm_psum.tile([90, BLK, HCH], F32, tag="mm")
                            nc.tensor.matmul(
                                ps[:], lhsT=cwi_sb[:, j, :],
                                rhs=tu[:, :, hs],
                                start=True, stop=True)
                            dst = (ysb[:, bass.ds(si * HCH, HCH), :]
                                   .rearrange("w h c -> w c h"))
                            if si % 2 == 0:
                                nc.vector.tensor_copy(dst, ps[:, :, :])
                            else:
                                nc.scalar.copy(dst, ps[:, :, :])
                        nc.sync.dma_start(
                            y_ext[p, j, :, h0g:h0g + HCH * YGRP, :], ysb[:])

    nc.compile()
    return nc


_CACHE = {}


def _get_graph():
    if "nc" not in _CACHE:
        _CACHE["nc"] = _build_graph()
    return _CACHE["nc"]


def kernel(x, w1, b1, w2, b2):
    x = np.ascontiguousarray(np.asarray(x, dtype=np.float32))
    w1 = np.asarray(w1, dtype=np.float32)
    b1 = np.asarray(b1, dtype=np.float32)
    w2 = np.asarray(w2, dtype=np.float32)
    b2 = np.asarray(b2, dtype=np.float32)

    wf8, rc, rs, cwi = _make_host_mats()
    nc = _get_graph()

    in_maps = []
    xcs = []
    for core in range(8):
        b = core // 2
        half = core % 2
        cs = half * CPC
        nb0 = half * NBPC
        xc = x[b, :, :, cs:cs + CPC]                      # [h, w, 384]
        xcs.append(xc)
        # [h, j, w', pass, c] -> [w', j, pass, h, c]
        x8 = np.ascontiguousarray(
            xc.reshape(H, 2, 90, NPASS, BLK).transpose(2, 1, 3, 0, 4)
        ).astype(E4)
        b2c = b2[0, nb0:nb0 + NBPC]                       # [NBPC, BLK]
        in_maps.append({
            "x8": x8,
            "wf8": wf8,
            "rc": rc,
            "rs": rs,
            "cwi": cwi,
            "w1b": w1[0, nb0:nb0 + NBPC].astype(BF),
            "w2b": w2[0, nb0:nb0 + NBPC].astype(BF),
            "b1b": np.ascontiguousarray(b1[0, nb0:nb0 + NBPC].T.astype(np.float32)),
            "b2m": np.ascontiguousarray((b2c - LAM).T.astype(np.float32)),
            "b2p": np.ascontiguousarray((-b2c - LAM).T.astype(np.float32)),
        })

    res = run_bass_kernel_spmd(nc, in_maps, core_ids=list(range(8)),
                               **_CACHE.get("run_kwargs", {}))
    _CACHE["last_result"] = res

    y = np.empty((B, H, W, C), np.float32)
    for core in range(8):
        b = core // 2
        cs = (core % 2) * CPC
        corr = res.results[core]["y"].astype(np.float32)  # [pass, j, w', h, c]
        # -> [h, j, w', pass, c] -> [h, w, 384]
        corr = corr.transpose(3, 1, 2, 0, 4).reshape(H, W, CPC)
        y[b, :, :, cs:cs + CPC] = xcs[core] + corr
    return y


if __name__ == "__main__":
    xs = np.random.randn(B, H, W, C).astype(np.float32)
    w1s = 0.02 * np.random.randn(2, NB, BLK, BLK).astype(np.float32)
    b1s = 0.02 * np.random.randn(2, NB, BLK).astype(np.float32)
    w2s = 0.02 * np.random.randn(2, NB, BLK, BLK).astype(np.float32)
    b2s = 0.02 * np.random.randn(2, NB, BLK).astype(np.float32)
    out = kernel(x=xs, w1=w1s, b1=b1s, w2=w2s, b2=b2s)
    print("ran, out shape", out.shape)


# revision 7
# speedup vs baseline: 1.1847x; 1.1847x over previous
"""AFNO2D Trainium2 kernel — 8 NeuronCores, no collectives.

Sharding: core = (b, c_half): b = core // 2, channels c_half*384 .. +384
(4 independent MLP blocks of 96 channels per core). Every stage is local.

The 2D Hartley transform (Re(FFT2) - Im(FFT2)) and its inverse are dense
matmuls against precomputed cos/sin matrices (H=90, W=180 fixed):
  MM1 (fp8 DoubleRow): P = PW @ x, M = MW @ x  (contract w=180 as [90,2])
  MM2: Xk = RC @ P + RS @ M                    (contract h)
  block-MLP per spectral point (2 layers, relu, softshrink)
  RI:  T = RC @ s, U = RS @ s                  (contract k1)
  WI:  corr = CWI @ [T;U]                      (contract m2-pad=128)

Partition-dim swaps between stages run as single batched DMA-crossbar
transposes (16x128 tiles), keeping the PE free for matmuls.

The residual add (y = x + corr) happens on the host: x is uploaded
pre-quantized to fp8 (only feeds the ~1%-magnitude spectral correction),
the correction is stored as bf16, and the f32 residual never loses
precision.
"""

import sys
import numpy as np

sys.path.insert(0, "/opt/trn_rl_repo")

import ml_dtypes  # noqa: E402
import concourse.bass as bass  # noqa: E402
import concourse.mybir as mybir  # noqa: E402
import concourse.tile as tile  # noqa: E402
from concourse import bacc  # noqa: E402
from concourse.bass_utils import run_bass_kernel_spmd  # noqa: E402

# problem constants (hardcoded per spec)
B, H, W, C = 4, 90, 180, 768
K2 = 46                 # kept width modes
NB, BLK = 8, 96
LAM = 0.01
CPC = 384               # channels per core
NBPC = 4                # MLP blocks per core
NPASS = 4               # c-passes of 96 channels (1 block) per core
HCH = 5                 # h-chunk -> N=480 for MM1/WI
CCH = 10                # c-chunk -> N=460 for MM2
K2CH = 5                # k2-chunk -> N=450 (MLP) / 480 (RI)
YGRP = 3                # WI h-chunks batched per store DMA
MU = 64                 # free-dim offset of the M/U halves (m2 hi part)

F32 = mybir.dt.float32
BF16 = mybir.dt.bfloat16
FP8 = mybir.dt.float8e4
BF = ml_dtypes.bfloat16
E4 = ml_dtypes.float8_e4m3


def _make_host_mats():
    w = np.arange(W)
    h = np.arange(H)
    k2 = np.arange(K2)
    beta = 2 * np.pi * np.outer(k2, w) / W            # [K2, W]
    PW = (np.cos(beta) + np.sin(beta)) / np.sqrt(W)
    MW = (np.cos(beta) - np.sin(beta)) / np.sqrt(W)
    # MM1 DoubleRow lhsT: [P/M, w', j, 48] (col 46,47 zero pad)
    wf8 = np.zeros((2, 90, 2, 48), np.float32)
    for j in range(2):
        wf8[0, :, j, :K2] = PW[:, j * 90:(j + 1) * 90].T
        wf8[1, :, j, :K2] = MW[:, j * 90:(j + 1) * 90].T
    alpha = 2 * np.pi * np.outer(h, h) / H            # [H, H] symmetric
    RC = np.cos(alpha) / np.sqrt(H)
    RS = np.sin(alpha) / np.sqrt(H)
    # WI lhsT [128, j, w']: rows k2 = cos, rows 64+k2 = -sin, rest zero
    cwi = np.zeros((128, 2, 90), np.float32)
    for j in range(2):
        ww = w[j * 90:(j + 1) * 90]
        bb = 2 * np.pi * np.outer(k2, ww) / W
        cwi[:K2, j, :] = np.cos(bb) / np.sqrt(W)
        cwi[MU:MU + K2, j, :] = -np.sin(bb) / np.sqrt(W)
    return (wf8.astype(E4), RC.astype(BF), RS.astype(BF), cwi.astype(BF))


def _build_graph():
    nc = bacc.Bacc("TRN2", target_bir_lowering=False, debug=False, num_devices=8)

    x8_ext = nc.declare_dram_parameter("x8", [90, 2, NPASS, H, BLK], FP8,
                                       isOutput=False)
    y_ext = nc.declare_dram_parameter("y", [NPASS, 2, 90, H, BLK], BF16,
                                      isOutput=True)
    wf8_ext = nc.declare_dram_parameter("wf8", [2, 90, 2, 48], FP8, isOutput=False)
    rc_ext = nc.declare_dram_parameter("rc", [H, H], BF16, isOutput=False)
    rs_ext = nc.declare_dram_parameter("rs", [H, H], BF16, isOutput=False)
    cwi_ext = nc.declare_dram_parameter("cwi", [128, 2, 90], BF16, isOutput=False)
    w1_ext = nc.declare_dram_parameter("w1b", [NBPC, BLK, BLK], BF16, isOutput=False)
    w2_ext = nc.declare_dram_parameter("w2b", [NBPC, BLK, BLK], BF16, isOutput=False)
    b1_ext = nc.declare_dram_parameter("b1b", [BLK, NBPC], F32, isOutput=False)
    # b2 folded with softshrink: b2m = b2 - lam (relu path 1), b2p = -b2 - lam
    b2m_ext = nc.declare_dram_parameter("b2m", [BLK, NBPC], F32, isOutput=False)
    b2p_ext = nc.declare_dram_parameter("b2p", [BLK, NBPC], F32, isOutput=False)

    DR = mybir.MatmulPerfMode.DoubleRow

    with tile.TileContext(nc) as tc:
        with (
            tc.tile_pool(name="consts", bufs=1) as consts,
            tc.tile_pool(name="x8", bufs=2) as x8_pool,
            tc.tile_pool(name="mid", bufs=1) as mid,
            tc.tile_pool(name="shr", bufs=2) as shr_pool,
            tc.tile_pool(name="ytile", bufs=3) as y_pool,
            tc.tile_pool(name="mmps", bufs=4, space="PSUM") as mm_psum,
        ):
            # ---- constants to SBUF ----
            wfP_sb = consts.tile([90, 2, 48], FP8)
            nc.sync.dma_start(wfP_sb[:], wf8_ext[0])
            wfM_sb = consts.tile([90, 2, 48], FP8)
            nc.sync.dma_start(wfM_sb[:], wf8_ext[1])
            rc_sb = consts.tile([H, H], BF16)
            nc.sync.dma_start(rc_sb[:], rc_ext[:])
            rs_sb = consts.tile([H, H], BF16)
            nc.sync.dma_start(rs_sb[:], rs_ext[:])
            cwi_sb = consts.tile([128, 2, 90], BF16)
            nc.sync.dma_start(cwi_sb[:], cwi_ext[:])
            w1_sb = consts.tile([BLK, NBPC, BLK], BF16)
            nc.sync.dma_start(w1_sb[:], w1_ext[:].rearrange("n i o -> i n o"))
            w2_sb = consts.tile([BLK, NBPC, BLK], BF16)
            nc.sync.dma_start(w2_sb[:], w2_ext[:].rearrange("n i o -> i n o"))
            b1_sb = consts.tile([BLK, NBPC], F32)
            nc.sync.dma_start(b1_sb[:], b1_ext[:])
            b2m_sb = consts.tile([BLK, NBPC], F32)
            nc.sync.dma_start(b2m_sb[:], b2m_ext[:])
            b2p_sb = consts.tile([BLK, NBPC], F32)
            nc.sync.dma_start(b2p_sb[:], b2p_ext[:])

            # ---- persistent intermediates (single-buffered) ----
            s1 = mid.tile([112, BLK, 128], BF16)      # [m2lay, c, h]
            s1t = mid.tile([128, BLK, 112], BF16)     # [h, c, m2lay]
            xk = mid.tile([96, K2, 128], BF16)        # [k1, k2, c]
            xkt = mid.tile([128, K2, 96], BF16)       # [c, k2, k1]
            o1 = mid.tile([BLK, K2, H], BF16)         # [c, k2, k1]
            o2s = mid.tile([BLK, K2, 128], BF16)      # [c, k2, k1pad]
            s3 = mid.tile([128, K2, BLK], BF16)       # [k1pad, k2, c]
            ttut = mid.tile([96, BLK, 128], BF16)     # [h, c, m2lay]
            tu = mid.tile([128, BLK, 96], BF16)       # [m2lay, c, h]
            # WI contracts all 128 tu rows against cwi (zero rows at the
            # pads); junk there must be 0.0, not NaN.
            nc.vector.memset(ttut[:, :, K2:MU], 0.0)
            nc.vector.memset(ttut[:, :, MU + K2:128], 0.0)
            # junk h-cols of o2s feed the k1 pad rows of s3 (never read)
            nc.vector.memset(o2s[:, :, H:128], 0.0)

            x8s = []
            for p in range(NPASS):
                t = x8_pool.tile([90, 2, H, BLK], FP8, tag="x8")
                x8s.append(t)
            nc.sync.dma_start(x8s[0][:], x8_ext[:, :, 0, :, :])

            for p in range(NPASS):
                x8t = x8s[p]
                if p + 1 < NPASS:
                    nc.sync.dma_start(x8s[p + 1][:], x8_ext[:, :, p + 1, :, :])

                # ---- MM1 (fp8 DR): contract w, out P/M halves ----
                for hi in range(H // HCH):
                    hs = slice(hi * HCH, (hi + 1) * HCH)
                    rhs = x8t[:, :, hs, :].rearrange("w j h c -> w j c h")
                    psP = mm_psum.tile([48, BLK, HCH], F32, tag="mm")
                    nc.tensor.matmul(psP[:], lhsT=wfP_sb[:], rhs=rhs,
                                     start=True, stop=True, perf_mode=DR)
                    psM = mm_psum.tile([48, BLK, HCH], F32, tag="mm")
                    nc.tensor.matmul(psM[:], lhsT=wfM_sb[:], rhs=rhs,
                                     start=True, stop=True, perf_mode=DR)
                    dstP = s1[0:K2, :, hs]
                    dstM = s1[MU:MU + K2, :, hs]
                    if hi % 2 == 0:
                        nc.scalar.copy(dstP, psP[0:K2, :, :])
                        nc.vector.tensor_copy(dstM, psM[0:K2, :, :])
                    else:
                        nc.vector.tensor_copy(dstP, psP[0:K2, :, :])
                        nc.scalar.copy(dstM, psM[0:K2, :, :])

                # ---- T1 (xbar): [m2lay, c, h] -> [h, c, m2lay] ----
                nc.scalar.dma_start(s1t[:], s1[:].rearrange("p c h -> p (c h)"),
                                    transpose=True)

                # ---- MM2: contract h; Xk = RC@P + RS@M, out [k1, c, k2] ----
                ncch = (BLK + CCH - 1) // CCH
                for ci in range(ncch):
                    c0 = ci * CCH
                    cn = min(CCH, BLK - c0)
                    cs = bass.ds(c0, cn)
                    ps = mm_psum.tile([H, K2, CCH], F32, tag="mm")
                    nc.tensor.matmul(
                        ps[:, :, 0:cn], lhsT=rc_sb[:],
                        rhs=s1t[0:H, cs, 0:K2].rearrange("h c m -> h m c"),
                        start=True, stop=False)
                    nc.tensor.matmul(
                        ps[:, :, 0:cn], lhsT=rs_sb[:],
                        rhs=s1t[0:H, cs, MU:MU + K2].rearrange("h c m -> h m c"),
                        start=False, stop=True)
                    dst = xk[0:H, :, cs]
                    if ci % 2 == 0:
                        nc.scalar.copy(dst, ps[:, :, 0:cn])
                    else:
                        nc.vector.tensor_copy(dst, ps[:, :, 0:cn])

                # ---- T2 (xbar): [k1, k2, c] -> [c, k2, k1] ----
                nc.sync.dma_start(xkt[:], xk[:].rearrange("p k c -> p (k c)"),
                                    transpose=True)

                # ---- MLP layer 1 + bias + relu ----
                nk2 = (K2 + K2CH - 1) // K2CH
                for ki in range(nk2):
                    k0 = ki * K2CH
                    kn = min(K2CH, K2 - k0)
                    ks = bass.ds(k0, kn)
                    ps = mm_psum.tile([BLK, K2CH, H], F32, tag="mm")
                    pss = ps[:, 0:kn, :]
                    nc.tensor.matmul(
                        pss, lhsT=w1_sb[:, p, :],
                        rhs=xkt[0:96, ks, 0:H],
                        start=True, stop=True)
                    nc.scalar.activation(
                        o1[:, ks, :],
                        pss, mybir.ActivationFunctionType.Relu,
                        bias=b1_sb[:, p:p + 1])

                # ---- MLP layer 2 + bias + softshrink ----
                for ki in range(nk2):
                    k0 = ki * K2CH
                    kn = min(K2CH, K2 - k0)
                    ks = bass.ds(k0, kn)
                    ps = mm_psum.tile([BLK, K2CH, H], F32, tag="mm")
                    pss = ps[:, 0:kn, :]
                    nc.tensor.matmul(
                        pss, lhsT=w2_sb[:, p, :],
                        rhs=o1[:, ks, :],
                        start=True, stop=True)
                    # softshrink(v + b2) = relu(v + b2 - lam) - relu(-v - b2 - lam)
                    sp = shr_pool.tile([BLK, K2CH, H], BF16, tag="shr_p")
                    sn = shr_pool.tile([BLK, K2CH, H], BF16, tag="shr_n")
                    nc.scalar.activation(
                        sp[:, 0:kn, :], pss, mybir.ActivationFunctionType.Relu,
                        bias=b2m_sb[:, p:p + 1], scale=1.0)
                    nc.scalar.activation(
                        sn[:, 0:kn, :], pss, mybir.ActivationFunctionType.Relu,
                        bias=b2p_sb[:, p:p + 1], scale=-1.0)
                    nc.vector.tensor_sub(
                        o2s[:, ks, 0:H],
                        sp[:, 0:kn, :], sn[:, 0:kn, :])

                # ---- T3 (xbar): [c, k2, k1pad] -> [k1pad, k2, c] ----
                nc.scalar.dma_start(s3[:], o2s[:].rearrange("c k i -> c (k i)"),
                                    transpose=True)

                # ---- RI: contract k1; T = RC@s3, U = RS@s3 ----
                for ki in range(nk2):
                    k0 = ki * K2CH
                    kn = min(K2CH, K2 - k0)
                    rhs = s3[0:H, bass.ds(k0, kn), :].rearrange("p k c -> p c k")
                    psT = mm_psum.tile([H, BLK, K2CH], F32, tag="mm")
                    nc.tensor.matmul(psT[:, :, 0:kn],
                                     lhsT=rc_sb[:], rhs=rhs,
                                     start=True, stop=True)
                    nc.scalar.copy(
                        ttut[0:H, :, bass.ds(k0, kn)],
                        psT[:, :, 0:kn])
                    psU = mm_psum.tile([H, BLK, K2CH], F32, tag="mm")
                    nc.tensor.matmul(psU[:, :, 0:kn],
                                     lhsT=rs_sb[:], rhs=rhs,
                                     start=True, stop=True)
                    nc.vector.tensor_copy(
                        ttut[0:H, :, bass.ds(MU + k0, kn)],
                        psU[:, :, 0:kn])

                # ---- T4 (xbar): [h, c, m2lay] -> [m2lay, c, h] ----
                nc.sync.dma_start(tu[:], ttut[:].rearrange("h c m -> h (c m)"),
                                    transpose=True)

                # ---- WI: contract m2 (128-pad), store bf16 correction ----
                for j in range(2):
                    for hg in range(H // (HCH * YGRP)):
                        h0g = hg * HCH * YGRP
                        ysb = y_pool.tile([90, HCH * YGRP, BLK], BF16, tag="y")
                        for si in range(YGRP):
                            h0 = h0g + si * HCH
                            hs = slice(h0, h0 + HCH)
                            ps = mm_psum.tile([90, HCH, BLK], F32, tag="mm")
                            nc.tensor.matmul(
                                ps[:], lhsT=cwi_sb[:, j, :],
                                rhs=tu[:, :, hs].rearrange("m c h -> m h c"),
                                start=True, stop=True)
                            dst = ysb[:, bass.ds(si * HCH, HCH), :]
                            if si % 2 == 0:
                                nc.vector.tensor_copy(dst, ps[:, :, :])
                            else:
                                nc.scalar.copy(dst, ps[:, :, :])
                        nc.sync.dma_start(
                            y_ext[p, j, :, h0g:h0g + HCH * YGRP, :], ysb[:])

    nc.compile()
    return nc


_CACHE = {}


def _get_graph():
    if "nc" not in _CACHE:
        _CACHE["nc"] = _build_graph()
    return _CACHE["nc"]


def kernel(x, w1, b1, w2, b2):
    x = np.ascontiguousarray(np.asarray(x, dtype=np.float32))
    w1 = np.asarray(w1, dtype=np.float32)
    b1 = np.asarray(b1, dtype=np.float32)
    w2 = np.asarray(w2, dtype=np.float32)
    b2 = np.asarray(b2, dtype=np.float32)

    wf8, rc, rs, cwi = _make_host_mats()
    nc = _get_graph()

    in_maps = []
    xcs = []
    for core in range(8):
        b = core // 2
        half = core % 2
        cs = half * CPC
        nb0 = half * NBPC
        xc = x[b, :, :, cs:cs + CPC]                      # [h, w, 384]
        xcs.append(xc)
        # [h, j, w', pass, c] -> [w', j, pass, h, c]
        x8 = np.ascontiguousarray(
            xc.reshape(H, 2, 90, NPASS, BLK).transpose(2, 1, 3, 0, 4)
        ).astype(E4)
        b2c = b2[0, nb0:nb0 + NBPC]                       # [NBPC, BLK]
        in_maps.append({
            "x8": x8,
            "wf8": wf8,
            "rc": rc,
            "rs": rs,
            "cwi": cwi,
            "w1b": w1[0, nb0:nb0 + NBPC].astype(BF),
            "w2b": w2[0, nb0:nb0 + NBPC].astype(BF),
            "b1b": np.ascontiguousarray(b1[0, nb0:nb0 + NBPC].T.astype(np.float32)),
            "b2m": np.ascontiguousarray((b2c - LAM).T.astype(np.float32)),
            "b2p": np.ascontiguousarray((-b2c - LAM).T.astype(np.float32)),
        })

    res = run_bass_kernel_spmd(nc, in_maps, core_ids=list(range(8)),
                               **_CACHE.get("run_kwargs", {}))
    _CACHE["last_result"] = res

    y = np.empty((B, H, W, C), np.float32)
    for core in range(8):
        b = core // 2
        cs = (core % 2) * CPC
        corr = res.results[core]["y"].astype(np.float32)  # [pass, j, w', h, c]
        # -> [h, j, w', pass, c] -> [h, w, 384]
        corr = corr.transpose(3, 1, 2, 0, 4).reshape(H, W, CPC)
        y[b, :, :, cs:cs + CPC] = xcs[core] + corr
    return y


if __name__ == "__main__":
    xs = np.random.randn(B, H, W, C).astype(np.float32)
    w1s = 0.02 * np.random.randn(2, NB, BLK, BLK).astype(np.float32)
    b1s = 0.02 * np.random.randn(2, NB, BLK).astype(np.float32)
    w2s = 0.02 * np.random.randn(2, NB, BLK, BLK).astype(np.float32)
    b2s = 0.02 * np.random.randn(2, NB, BLK).astype(np.float32)
    out = kernel(x=xs, w1=w1s, b1=b1s, w2=w2s, b2=b2s)
    print("ran, out shape", out.shape)


# revision 13
# speedup vs baseline: 1.3077x; 1.1038x over previous
"""AFNO2D Trainium2 kernel — 8 NeuronCores, no collectives.

Sharding: core = (b, c_half): b = core // 2, channels c_half*384 .. +384
(4 independent MLP blocks of 96 channels per core). Every stage is local.

The 2D Hartley transform (Re(FFT2) - Im(FFT2)) and its inverse are dense
matmuls against precomputed cos/sin matrices (H=90, W=180 fixed):
  MM1 (fp8 DoubleRow): P = PW @ x, M = MW @ x  (contract w=180 as [90,2])
  MM2: Xk = RC @ P + RS @ M                    (contract h)
  block-MLP per spectral point (2 layers, relu, softshrink)
  RI:  T = RC @ s, U = RS @ s                  (contract k1)
  WI:  corr = CWI @ [T;U]                      (contract m2-pad=128)

Partition-dim swaps between stages run as single batched DMA-crossbar
transposes (16x128 tiles), keeping the PE free for matmuls.

The residual add (y = x + corr) happens on the host: x is uploaded
pre-quantized to fp8 (only feeds the ~1%-magnitude spectral correction),
the correction is stored as bf16, and the f32 residual never loses
precision.
"""

import sys
import numpy as np

sys.path.insert(0, "/opt/trn_rl_repo")

import ml_dtypes  # noqa: E402
import concourse.bass as bass  # noqa: E402
import concourse.mybir as mybir  # noqa: E402
import concourse.tile as tile  # noqa: E402
from concourse import bacc  # noqa: E402
from concourse.bass_utils import run_bass_kernel_spmd  # noqa: E402

# problem constants (hardcoded per spec)
B, H, W, C = 4, 90, 180, 768
K2 = 46                 # kept width modes
NB, BLK = 8, 96
LAM = 0.01
CPC = 384               # channels per core
NBPC = 4                # MLP blocks per core
NPASS = 4               # c-passes of 96 channels (1 block) per core
HCH = 5                 # h-chunk -> N=480 for MM1/WI
CCH = 10                # c-chunk -> N=460 for MM2
K2CH = 5                # k2-chunk -> N=450 (MLP) / 480 (RI)
YGRP = 3                # WI h-chunks batched per store DMA
MU = 64                 # free-dim offset of the M/U halves (m2 hi part)

F32 = mybir.dt.float32
BF16 = mybir.dt.bfloat16
FP8 = mybir.dt.float8e4
BF = ml_dtypes.bfloat16
E4 = ml_dtypes.float8_e4m3


def _make_host_mats():
    w = np.arange(W)
    h = np.arange(H)
    k2 = np.arange(K2)
    beta = 2 * np.pi * np.outer(k2, w) / W            # [K2, W]
    PW = (np.cos(beta) + np.sin(beta)) / np.sqrt(W)
    MW = (np.cos(beta) - np.sin(beta)) / np.sqrt(W)
    # MM1 DoubleRow lhsT: [P/M, w', j, 48] (col 46,47 zero pad)
    wf8 = np.zeros((2, 90, 2, 48), np.float32)
    for j in range(2):
        wf8[0, :, j, :K2] = PW[:, j * 90:(j + 1) * 90].T
        wf8[1, :, j, :K2] = MW[:, j * 90:(j + 1) * 90].T
    alpha = 2 * np.pi * np.outer(h, h) / H            # [H, H] symmetric
    RC = np.cos(alpha) / np.sqrt(H)
    RS = np.sin(alpha) / np.sqrt(H)
    # WI lhsT [128, j, w']: rows k2 = cos, rows 64+k2 = -sin, rest zero
    cwi = np.zeros((128, 2, 90), np.float32)
    for j in range(2):
        ww = w[j * 90:(j + 1) * 90]
        bb = 2 * np.pi * np.outer(k2, ww) / W
        cwi[:K2, j, :] = np.cos(bb) / np.sqrt(W)
        cwi[MU:MU + K2, j, :] = -np.sin(bb) / np.sqrt(W)
    return (wf8.astype(E4), RC.astype(BF), RS.astype(BF), cwi.astype(BF))


def _build_graph():
    nc = bacc.Bacc("TRN2", target_bir_lowering=False, debug=False, num_devices=8)

    x8_ext = nc.declare_dram_parameter("x8", [90, 2, NPASS, H, BLK], FP8,
                                       isOutput=False)
    y_ext = nc.declare_dram_parameter("y", [NPASS, 2, 90, H, BLK], BF16,
                                      isOutput=True)
    wf8_ext = nc.declare_dram_parameter("wf8", [2, 90, 2, 48], FP8, isOutput=False)
    rc_ext = nc.declare_dram_parameter("rc", [H, H], BF16, isOutput=False)
    rs_ext = nc.declare_dram_parameter("rs", [H, H], BF16, isOutput=False)
    cwi_ext = nc.declare_dram_parameter("cwi", [128, 2, 90], BF16, isOutput=False)
    w1_ext = nc.declare_dram_parameter("w1b", [NBPC, BLK, BLK], BF16, isOutput=False)
    w2_ext = nc.declare_dram_parameter("w2b", [NBPC, BLK, BLK], BF16, isOutput=False)
    b1_ext = nc.declare_dram_parameter("b1b", [BLK, NBPC], F32, isOutput=False)
    # b2 folded with softshrink: b2m = b2 - lam (relu path 1), b2p = -b2 - lam
    b2m_ext = nc.declare_dram_parameter("b2m", [BLK, NBPC], F32, isOutput=False)
    b2p_ext = nc.declare_dram_parameter("b2p", [BLK, NBPC], F32, isOutput=False)

    DR = mybir.MatmulPerfMode.DoubleRow

    with tile.TileContext(nc) as tc:
        with (
            tc.tile_pool(name="consts", bufs=1) as consts,
            tc.tile_pool(name="x8", bufs=2) as x8_pool,
            tc.tile_pool(name="mid", bufs=1) as mid,
            tc.tile_pool(name="shr", bufs=2) as shr_pool,
            tc.tile_pool(name="ytile", bufs=3) as y_pool,
            tc.tile_pool(name="mmps", bufs=4, space="PSUM") as mm_psum,
        ):
            # ---- constants to SBUF ----
            wfP_sb = consts.tile([90, 2, 48], FP8)
            nc.sync.dma_start(wfP_sb[:], wf8_ext[0])
            wfM_sb = consts.tile([90, 2, 48], FP8)
            nc.sync.dma_start(wfM_sb[:], wf8_ext[1])
            rc_sb = consts.tile([H, H], BF16)
            nc.sync.dma_start(rc_sb[:], rc_ext[:])
            rs_sb = consts.tile([H, H], BF16)
            nc.sync.dma_start(rs_sb[:], rs_ext[:])
            cwi_sb = consts.tile([128, 2, 90], BF16)
            nc.sync.dma_start(cwi_sb[:], cwi_ext[:])
            w1_sb = consts.tile([BLK, NBPC, BLK], BF16)
            nc.sync.dma_start(w1_sb[:], w1_ext[:].rearrange("n i o -> i n o"))
            w2_sb = consts.tile([BLK, NBPC, BLK], BF16)
            nc.sync.dma_start(w2_sb[:], w2_ext[:].rearrange("n i o -> i n o"))
            b1_sb = consts.tile([BLK, NBPC], F32)
            nc.sync.dma_start(b1_sb[:], b1_ext[:])
            b2m_sb = consts.tile([BLK, NBPC], F32)
            nc.sync.dma_start(b2m_sb[:], b2m_ext[:])
            b2p_sb = consts.tile([BLK, NBPC], F32)
            nc.sync.dma_start(b2p_sb[:], b2p_ext[:])

            # ---- persistent intermediates (single-buffered) ----
            s1 = mid.tile([112, BLK, 128], BF16)      # [m2lay, c, h]
            s1t = mid.tile([128, BLK, 112], BF16)     # [h, c, m2lay]
            xk = mid.tile([96, K2, 128], BF16)        # [k1, k2, c]
            xkt = mid.tile([128, K2, 96], BF16)       # [c, k2, k1]
            o1 = mid.tile([BLK, K2, H], BF16)         # [c, k2, k1]
            o2s = mid.tile([BLK, K2, 128], BF16)      # [c, k2, k1pad]
            s3 = mid.tile([128, K2, BLK], BF16)       # [k1pad, k2, c]
            ttut = mid.tile([96, BLK, 128], BF16)     # [h, c, m2lay]
            tu = mid.tile([128, BLK, 96], BF16)       # [m2lay, c, h]
            # WI contracts all 128 tu rows against cwi (zero rows at the
            # pads); junk there must be 0.0, not NaN.
            nc.vector.memset(ttut[:, :, K2:MU], 0.0)
            nc.vector.memset(ttut[:, :, MU + K2:128], 0.0)
            # junk h-cols of o2s feed the k1 pad rows of s3 (never read)
            nc.vector.memset(o2s[:, :, H:128], 0.0)

            x8s = []
            for p in range(NPASS):
                t = x8_pool.tile([90, 2, H, BLK], FP8, tag="x8")
                x8s.append(t)
            nc.sync.dma_start(x8s[0][:], x8_ext[:, :, 0, :, :])

            for p in range(NPASS):
                x8t = x8s[p]
                if p + 1 < NPASS:
                    nc.sync.dma_start(x8s[p + 1][:], x8_ext[:, :, p + 1, :, :])

                # ---- MM1 (fp8 DR): contract w, out P/M halves ----
                for hi in range(H // HCH):
                    hs = slice(hi * HCH, (hi + 1) * HCH)
                    rhs = x8t[:, :, hs, :].rearrange("w j h c -> w j c h")
                    psP = mm_psum.tile([48, BLK, HCH], F32, tag="mm")
                    nc.tensor.matmul(psP[:], lhsT=wfP_sb[:], rhs=rhs,
                                     start=True, stop=True, perf_mode=DR)
                    psM = mm_psum.tile([48, BLK, HCH], F32, tag="mm")
                    nc.tensor.matmul(psM[:], lhsT=wfM_sb[:], rhs=rhs,
                                     start=True, stop=True, perf_mode=DR)
                    dstP = s1[0:K2, :, hs]
                    dstM = s1[MU:MU + K2, :, hs]
                    if hi % 2 == 0:
                        nc.scalar.copy(dstP, psP[0:K2, :, :])
                        nc.vector.tensor_copy(dstM, psM[0:K2, :, :])
                    else:
                        nc.vector.tensor_copy(dstP, psP[0:K2, :, :])
                        nc.scalar.copy(dstM, psM[0:K2, :, :])

                # ---- T1 (xbar): [m2lay, c, h] -> [h, c, m2lay] ----
                for g in range(4):
                    cg = bass.ds(g * 24, 24)
                    nc.scalar.dma_start(s1t[:, cg, :],
                                        s1[:, cg, :].rearrange("p c h -> p (c h)"),
                                        transpose=True)

                # ---- MM2: contract h; Xk = RC@P + RS@M, out [k1, c, k2] ----
                ncch = (BLK + CCH - 1) // CCH
                for ci in range(ncch):
                    c0 = ci * CCH
                    cn = min(CCH, BLK - c0)
                    cs = bass.ds(c0, cn)
                    ps = mm_psum.tile([H, K2, CCH], F32, tag="mm")
                    nc.tensor.matmul(
                        ps[:, :, 0:cn], lhsT=rc_sb[:],
                        rhs=s1t[0:H, cs, 0:K2].rearrange("h c m -> h m c"),
                        start=True, stop=False)
                    nc.tensor.matmul(
                        ps[:, :, 0:cn], lhsT=rs_sb[:],
                        rhs=s1t[0:H, cs, MU:MU + K2].rearrange("h c m -> h m c"),
                        start=False, stop=True)
                    dst = xk[0:H, :, cs]
                    if ci % 2 == 0:
                        nc.scalar.copy(dst, ps[:, :, 0:cn])
                    else:
                        nc.vector.tensor_copy(dst, ps[:, :, 0:cn])

                # ---- T2 (xbar): [k1, k2, c] -> [c, k2, k1] ----
                for g in range(2):
                    kg = bass.ds(g * 23, 23)
                    nc.sync.dma_start(xkt[:, kg, :],
                                      xk[:, kg, :].rearrange("p k c -> p (k c)"),
                                      transpose=True)

                # ---- MLP layer 1 + bias + relu ----
                nk2 = (K2 + K2CH - 1) // K2CH
                for ki in range(nk2):
                    k0 = ki * K2CH
                    kn = min(K2CH, K2 - k0)
                    ks = bass.ds(k0, kn)
                    ps = mm_psum.tile([BLK, K2CH, H], F32, tag="mm")
                    pss = ps[:, 0:kn, :]
                    nc.tensor.matmul(
                        pss, lhsT=w1_sb[:, p, :],
                        rhs=xkt[0:96, ks, 0:H],
                        start=True, stop=True)
                    nc.scalar.activation(
                        o1[:, ks, :],
                        pss, mybir.ActivationFunctionType.Relu,
                        bias=b1_sb[:, p:p + 1])

                # ---- MLP layer 2 + bias + softshrink ----
                for ki in range(nk2):
                    k0 = ki * K2CH
                    kn = min(K2CH, K2 - k0)
                    ks = bass.ds(k0, kn)
                    ps = mm_psum.tile([BLK, K2CH, H], F32, tag="mm")
                    pss = ps[:, 0:kn, :]
                    nc.tensor.matmul(
                        pss, lhsT=w2_sb[:, p, :],
                        rhs=o1[:, ks, :],
                        start=True, stop=True)
                    # softshrink(v + b2) = relu(v + b2 - lam) - relu(-v - b2 - lam)
                    sp = shr_pool.tile([BLK, K2CH, H], BF16, tag="shr_p")
                    sn = shr_pool.tile([BLK, K2CH, H], BF16, tag="shr_n")
                    nc.scalar.activation(
                        sp[:, 0:kn, :], pss, mybir.ActivationFunctionType.Relu,
                        bias=b2m_sb[:, p:p + 1], scale=1.0)
                    nc.scalar.activation(
                        sn[:, 0:kn, :], pss, mybir.ActivationFunctionType.Relu,
                        bias=b2p_sb[:, p:p + 1], scale=-1.0)
                    nc.vector.tensor_sub(
                        o2s[:, ks, 0:H],
                        sp[:, 0:kn, :], sn[:, 0:kn, :])

                # ---- T3 (xbar): [c, k2, k1pad] -> [k1pad, k2, c] ----
                for g in range(2):
                    kg = bass.ds(g * 23, 23)
                    nc.scalar.dma_start(s3[:, kg, :],
                                        o2s[:, kg, :].rearrange("c k i -> c (k i)"),
                                        transpose=True)

                # ---- RI: contract k1; T = RC@s3, U = RS@s3 ----
                for ki in range(nk2):
                    k0 = ki * K2CH
                    kn = min(K2CH, K2 - k0)
                    rhs = s3[0:H, bass.ds(k0, kn), :].rearrange("p k c -> p c k")
                    psT = mm_psum.tile([H, BLK, K2CH], F32, tag="mm")
                    nc.tensor.matmul(psT[:, :, 0:kn],
                                     lhsT=rc_sb[:], rhs=rhs,
                                     start=True, stop=True)
                    nc.scalar.copy(
                        ttut[0:H, :, bass.ds(k0, kn)],
                        psT[:, :, 0:kn])
                    psU = mm_psum.tile([H, BLK, K2CH], F32, tag="mm")
                    nc.tensor.matmul(psU[:, :, 0:kn],
                                     lhsT=rs_sb[:], rhs=rhs,
                                     start=True, stop=True)
                    nc.vector.tensor_copy(
                        ttut[0:H, :, bass.ds(MU + k0, kn)],
                        psU[:, :, 0:kn])

                # ---- T4 (xbar): [h, c, m2lay] -> [m2lay, c, h] ----
                for g, (h0, hn) in enumerate(((0, 64), (64, 32))):
                    nc.sync.dma_start(tu[:, :, bass.ds(h0, hn)],
                                      ttut[h0:h0 + hn, :, :]
                                      .rearrange("h c m -> h (c m)"),
                                      transpose=True)

                # ---- WI: contract m2 (128-pad), store bf16 correction ----
                for j in range(2):
                    for hg in range(H // (HCH * YGRP)):
                        h0g = hg * HCH * YGRP
                        ysb = y_pool.tile([90, HCH * YGRP, BLK], BF16, tag="y")
                        for si in range(YGRP):
                            h0 = h0g + si * HCH
                            hs = slice(h0, h0 + HCH)
                            ps = mm_psum.tile([90, HCH, BLK], F32, tag="mm")
                            nc.tensor.matmul(
                                ps[:], lhsT=cwi_sb[:, j, :],
                                rhs=tu[:, :, hs].rearrange("m c h -> m h c"),
                                start=True, stop=True)
                            dst = ysb[:, bass.ds(si * HCH, HCH), :]
                            if si % 2 == 0:
                                nc.vector.tensor_copy(dst, ps[:, :, :])
                            else:
                                nc.scalar.copy(dst, ps[:, :, :])
                        nc.sync.dma_start(
                            y_ext[p, j, :, h0g:h0g + HCH * YGRP, :], ysb[:])

    nc.compile()
    return nc


_CACHE = {}


def _get_graph():
    if "nc" not in _CACHE:
        _CACHE["nc"] = _build_graph()
    return _CACHE["nc"]


def kernel(x, w1, b1, w2, b2):
    x = np.ascontiguousarray(np.asarray(x, dtype=np.float32))
    w1 = np.asarray(w1, dtype=np.float32)
    b1 = np.asarray(b1, dtype=np.float32)
    w2 = np.asarray(w2, dtype=np.float32)
    b2 = np.asarray(b2, dtype=np.float32)

    wf8, rc, rs, cwi = _make_host_mats()
    nc = _get_graph()

    in_maps = []
    xcs = []
    for core in range(8):
        b = core // 2
        half = core % 2
        cs = half * CPC
        nb0 = half * NBPC
        xc = x[b, :, :, cs:cs + CPC]                      # [h, w, 384]
        xcs.append(xc)
        # [h, j, w', pass, c] -> [w', j, pass, h, c]
        x8 = np.ascontiguousarray(
            xc.reshape(H, 2, 90, NPASS, BLK).transpose(2, 1, 3, 0, 4)
        ).astype(E4)
        b2c = b2[0, nb0:nb0 + NBPC]                       # [NBPC, BLK]
        in_maps.append({
            "x8": x8,
            "wf8": wf8,
            "rc": rc,
            "rs": rs,
            "cwi": cwi,
            "w1b": w1[0, nb0:nb0 + NBPC].astype(BF),
            "w2b": w2[0, nb0:nb0 + NBPC].astype(BF),
            "b1b": np.ascontiguousarray(b1[0, nb0:nb0 + NBPC].T.astype(np.float32)),
            "b2m": np.ascontiguousarray((b2c - LAM).T.astype(np.float32)),
            "b2p": np.ascontiguousarray((-b2c - LAM).T.astype(np.float32)),
        })

    res = run_bass_kernel_spmd(nc, in_maps, core_ids=list(range(8)),
                               **_CACHE.get("run_kwargs", {}))
    _CACHE["last_result"] = res

    y = np.empty((B, H, W, C), np.float32)
    for core in range(8):
        b = core // 2
        cs = (core % 2) * CPC
        corr = res.results[core]["y"].astype(np.float32)  # [pass, j, w', h, c]
        # -> [h, j, w', pass, c] -> [h, w, 384]
        corr = corr.transpose(3, 1, 2, 0, 4).reshape(H, W, CPC)
        y[b, :, :, cs:cs + CPC] = xcs[core] + corr
    return y


if __name__ == "__main__":
    xs = np.random.randn(B, H, W, C).astype(np.float32)
    w1s = 0.02 * np.random.randn(2, NB, BLK, BLK).astype(np.float32)
    b1s = 0.02 * np.random.randn(2, NB, BLK).astype(np.float32)
    w2s = 0.02 * np.random.randn(2, NB, BLK, BLK).astype(np.float32)
    b2s = 0.02 * np.random.randn(2, NB, BLK).astype(np.float32)
    out = kernel(x=xs, w1=w1s, b1=b1s, w2=w2s, b2=b2s)
    print("ran, out shape", out.shape)


# revision 14
# speedup vs baseline: 1.4328x; 1.0957x over previous
"""AFNO2D Trainium2 kernel — 8 NeuronCores, no collectives.

Sharding: core = (b, c_half): b = core // 2, channels c_half*384 .. +384
(4 independent MLP blocks of 96 channels per core). Every stage is local.

The 2D Hartley transform (Re(FFT2) - Im(FFT2)) and its inverse are dense
matmuls against precomputed cos/sin matrices (H=90, W=180 fixed):
  MM1 (fp8 DoubleRow): P = PW @ x, M = MW @ x  (contract w=180 as [90,2])
  MM2: Xk = RC @ P + RS @ M                    (contract h)
  block-MLP per spectral point (2 layers, relu, softshrink)
  RI:  T = RC @ s, U = RS @ s                  (contract k1)
  WI:  corr = CWI @ [T;U]                      (contract m2-pad=128)

Partition-dim swaps between stages run as single batched DMA-crossbar
transposes (16x128 tiles), keeping the PE free for matmuls.

The residual add (y = x + corr) happens on the host: x is uploaded
pre-quantized to fp8 (only feeds the ~1%-magnitude spectral correction),
the correction is stored as bf16, and the f32 residual never loses
precision.
"""

import sys
import numpy as np

sys.path.insert(0, "/opt/trn_rl_repo")

import ml_dtypes  # noqa: E402
import concourse.bass as bass  # noqa: E402
import concourse.mybir as mybir  # noqa: E402
import concourse.tile as tile  # noqa: E402
from concourse import bacc  # noqa: E402
from concourse.bass_utils import run_bass_kernel_spmd  # noqa: E402

# problem constants (hardcoded per spec)
B, H, W, C = 4, 90, 180, 768
K2 = 46                 # kept width modes
NB, BLK = 8, 96
LAM = 0.01
CPC = 384               # channels per core
NBPC = 4                # MLP blocks per core
NPASS = 4               # c-passes of 96 channels (1 block) per core
HCH = 5                 # h-chunk -> N=480 for MM1/WI
CCH = 10                # c-chunk -> N=460 for MM2
K2CH = 5                # k2-chunk -> N=450 (MLP) / 480 (RI)
YGRP = 3                # WI h-chunks batched per store DMA
MU = 64                 # free-dim offset of the M/U halves (m2 hi part)

F32 = mybir.dt.float32
BF16 = mybir.dt.bfloat16
FP8 = mybir.dt.float8e4
BF = ml_dtypes.bfloat16
E4 = ml_dtypes.float8_e4m3


def _make_host_mats():
    w = np.arange(W)
    h = np.arange(H)
    k2 = np.arange(K2)
    beta = 2 * np.pi * np.outer(k2, w) / W            # [K2, W]
    PW = (np.cos(beta) + np.sin(beta)) / np.sqrt(W)
    MW = (np.cos(beta) - np.sin(beta)) / np.sqrt(W)
    # MM1 DoubleRow lhsT: [P/M, w', j, 48] (col 46,47 zero pad)
    wf8 = np.zeros((2, 90, 2, 48), np.float32)
    for j in range(2):
        wf8[0, :, j, :K2] = PW[:, j * 90:(j + 1) * 90].T
        wf8[1, :, j, :K2] = MW[:, j * 90:(j + 1) * 90].T
    alpha = 2 * np.pi * np.outer(h, h) / H            # [H, H] symmetric
    RC = np.cos(alpha) / np.sqrt(H)
    RS = np.sin(alpha) / np.sqrt(H)
    # WI lhsT [128, j, w']: rows k2 = cos, rows 64+k2 = -sin, rest zero
    cwi = np.zeros((128, 2, 90), np.float32)
    for j in range(2):
        ww = w[j * 90:(j + 1) * 90]
        bb = 2 * np.pi * np.outer(k2, ww) / W
        cwi[:K2, j, :] = np.cos(bb) / np.sqrt(W)
        cwi[MU:MU + K2, j, :] = -np.sin(bb) / np.sqrt(W)
    return (wf8.astype(E4), RC.astype(BF), RS.astype(BF), cwi.astype(BF))


def _build_graph():
    nc = bacc.Bacc("TRN2", target_bir_lowering=False, debug=False, num_devices=8)

    x8_ext = nc.declare_dram_parameter("x8", [90, 2, NPASS, H, BLK], FP8,
                                       isOutput=False)
    y_ext = nc.declare_dram_parameter("y", [NPASS, 2, 90, H, BLK], BF16,
                                      isOutput=True)
    wf8_ext = nc.declare_dram_parameter("wf8", [2, 90, 2, 48], FP8, isOutput=False)
    rc_ext = nc.declare_dram_parameter("rc", [H, H], BF16, isOutput=False)
    rs_ext = nc.declare_dram_parameter("rs", [H, H], BF16, isOutput=False)
    cwi_ext = nc.declare_dram_parameter("cwi", [128, 2, 90], BF16, isOutput=False)
    w1_ext = nc.declare_dram_parameter("w1b", [NBPC, BLK, BLK], BF16, isOutput=False)
    w2_ext = nc.declare_dram_parameter("w2b", [NBPC, BLK, BLK], BF16, isOutput=False)
    b1_ext = nc.declare_dram_parameter("b1b", [BLK, NBPC], F32, isOutput=False)
    # b2 folded with softshrink: b2m = b2 - lam (relu path 1), b2p = -b2 - lam
    b2m_ext = nc.declare_dram_parameter("b2m", [BLK, NBPC], F32, isOutput=False)
    b2p_ext = nc.declare_dram_parameter("b2p", [BLK, NBPC], F32, isOutput=False)

    DR = mybir.MatmulPerfMode.DoubleRow

    with tile.TileContext(nc) as tc:
        with (
            tc.tile_pool(name="consts", bufs=1) as consts,
            tc.tile_pool(name="x8", bufs=2) as x8_pool,
            tc.tile_pool(name="mid", bufs=1) as mid,
            tc.tile_pool(name="shr", bufs=2) as shr_pool,
            tc.tile_pool(name="ytile", bufs=3) as y_pool,
            tc.tile_pool(name="mmps", bufs=4, space="PSUM") as mm_psum,
        ):
            # ---- constants to SBUF ----
            wfP_sb = consts.tile([90, 2, 48], FP8)
            nc.sync.dma_start(wfP_sb[:], wf8_ext[0])
            wfM_sb = consts.tile([90, 2, 48], FP8)
            nc.sync.dma_start(wfM_sb[:], wf8_ext[1])
            rc_sb = consts.tile([H, H], BF16)
            nc.sync.dma_start(rc_sb[:], rc_ext[:])
            rs_sb = consts.tile([H, H], BF16)
            nc.sync.dma_start(rs_sb[:], rs_ext[:])
            cwi_sb = consts.tile([128, 2, 90], BF16)
            nc.sync.dma_start(cwi_sb[:], cwi_ext[:])
            w1_sb = consts.tile([BLK, NBPC, BLK], BF16)
            nc.sync.dma_start(w1_sb[:], w1_ext[:].rearrange("n i o -> i n o"))
            w2_sb = consts.tile([BLK, NBPC, BLK], BF16)
            nc.sync.dma_start(w2_sb[:], w2_ext[:].rearrange("n i o -> i n o"))
            b1_sb = consts.tile([BLK, NBPC], F32)
            nc.sync.dma_start(b1_sb[:], b1_ext[:])
            b2m_sb = consts.tile([BLK, NBPC], F32)
            nc.sync.dma_start(b2m_sb[:], b2m_ext[:])
            b2p_sb = consts.tile([BLK, NBPC], F32)
            nc.sync.dma_start(b2p_sb[:], b2p_ext[:])

            # ---- persistent intermediates (single-buffered) ----
            s1 = mid.tile([112, BLK, 128], BF16)      # [m2lay, c, h]
            s1t = mid.tile([128, BLK, 112], BF16)     # [h, c, m2lay]
            xk = mid.tile([96, K2, 128], BF16)        # [k1, k2, c]
            xkt = mid.tile([128, K2, 96], BF16)       # [c, k2, k1]
            o1 = mid.tile([BLK, K2, H], BF16)         # [c, k2, k1]
            o2s = mid.tile([BLK, K2, 128], BF16)      # [c, k2, k1pad]
            s3 = mid.tile([128, K2, BLK], BF16)       # [k1pad, k2, c]
            ttut = mid.tile([96, BLK, 128], BF16)     # [h, c, m2lay]
            tu = mid.tile([128, BLK, 96], BF16)       # [m2lay, c, h]
            # WI contracts all 128 tu rows against cwi (zero rows at the
            # pads); junk there must be 0.0, not NaN.
            nc.vector.memset(ttut[:, :, K2:MU], 0.0)
            nc.vector.memset(ttut[:, :, MU + K2:128], 0.0)
            # junk h-cols of o2s feed the k1 pad rows of s3 (never read)
            nc.vector.memset(o2s[:, :, H:128], 0.0)

            x8s = []
            for p in range(NPASS):
                t = x8_pool.tile([90, 2, H, BLK], FP8, tag="x8")
                x8s.append(t)
            nc.sync.dma_start(x8s[0][:], x8_ext[:, :, 0, :, :])

            def stage_mm1(p):
                x8t = x8s[p]
                if p + 1 < NPASS:
                    nc.sync.dma_start(x8s[p + 1][:], x8_ext[:, :, p + 1, :, :])
                for hi in range(H // HCH):
                    hs = slice(hi * HCH, (hi + 1) * HCH)
                    rhs = x8t[:, :, hs, :].rearrange("w j h c -> w j c h")
                    psP = mm_psum.tile([48, BLK, HCH], F32, tag="mm")
                    nc.tensor.matmul(psP[:], lhsT=wfP_sb[:], rhs=rhs,
                                     start=True, stop=True, perf_mode=DR)
                    psM = mm_psum.tile([48, BLK, HCH], F32, tag="mm")
                    nc.tensor.matmul(psM[:], lhsT=wfM_sb[:], rhs=rhs,
                                     start=True, stop=True, perf_mode=DR)
                    dstP = s1[0:K2, :, hs]
                    dstM = s1[MU:MU + K2, :, hs]
                    if hi % 2 == 0:
                        nc.scalar.copy(dstP, psP[0:K2, :, :])
                        nc.vector.tensor_copy(dstM, psM[0:K2, :, :])
                    else:
                        nc.vector.tensor_copy(dstP, psP[0:K2, :, :])
                        nc.scalar.copy(dstM, psM[0:K2, :, :])

            def stage_t1(p):
                for g in range(4):
                    cg = bass.ds(g * 24, 24)
                    nc.scalar.dma_start(s1t[:, cg, :],
                                        s1[:, cg, :].rearrange("p c h -> p (c h)"),
                                        transpose=True)

            def stage_mm2(p):
                ncch = (BLK + CCH - 1) // CCH
                for ci in range(ncch):
                    c0 = ci * CCH
                    cn = min(CCH, BLK - c0)
                    cs = bass.ds(c0, cn)
                    ps = mm_psum.tile([H, K2, CCH], F32, tag="mm")
                    nc.tensor.matmul(
                        ps[:, :, 0:cn], lhsT=rc_sb[:],
                        rhs=s1t[0:H, cs, 0:K2].rearrange("h c m -> h m c"),
                        start=True, stop=False)
                    nc.tensor.matmul(
                        ps[:, :, 0:cn], lhsT=rs_sb[:],
                        rhs=s1t[0:H, cs, MU:MU + K2].rearrange("h c m -> h m c"),
                        start=False, stop=True)
                    dst = xk[0:H, :, cs]
                    if ci % 2 == 0:
                        nc.scalar.copy(dst, ps[:, :, 0:cn])
                    else:
                        nc.vector.tensor_copy(dst, ps[:, :, 0:cn])

            def stage_t2(p):
                for g in range(2):
                    kg = bass.ds(g * 23, 23)
                    nc.sync.dma_start(xkt[:, kg, :],
                                      xk[:, kg, :].rearrange("p k c -> p (k c)"),
                                      transpose=True)

            def stage_mlp(p):
                nk2 = (K2 + K2CH - 1) // K2CH
                for ki in range(nk2):
                    k0 = ki * K2CH
                    kn = min(K2CH, K2 - k0)
                    ks = bass.ds(k0, kn)
                    ps = mm_psum.tile([BLK, K2CH, H], F32, tag="mm")
                    pss = ps[:, 0:kn, :]
                    nc.tensor.matmul(
                        pss, lhsT=w1_sb[:, p, :],
                        rhs=xkt[0:96, ks, 0:H],
                        start=True, stop=True)
                    nc.scalar.activation(
                        o1[:, ks, :],
                        pss, mybir.ActivationFunctionType.Relu,
                        bias=b1_sb[:, p:p + 1])
                for ki in range(nk2):
                    k0 = ki * K2CH
                    kn = min(K2CH, K2 - k0)
                    ks = bass.ds(k0, kn)
                    ps = mm_psum.tile([BLK, K2CH, H], F32, tag="mm")
                    pss = ps[:, 0:kn, :]
                    nc.tensor.matmul(
                        pss, lhsT=w2_sb[:, p, :],
                        rhs=o1[:, ks, :],
                        start=True, stop=True)
                    # softshrink(v + b2) = relu(v + b2 - lam) - relu(-v - b2 - lam)
                    sp = shr_pool.tile([BLK, K2CH, H], BF16, tag="shr_p")
                    sn = shr_pool.tile([BLK, K2CH, H], BF16, tag="shr_n")
                    nc.scalar.activation(
                        sp[:, 0:kn, :], pss, mybir.ActivationFunctionType.Relu,
                        bias=b2m_sb[:, p:p + 1], scale=1.0)
                    nc.scalar.activation(
                        sn[:, 0:kn, :], pss, mybir.ActivationFunctionType.Relu,
                        bias=b2p_sb[:, p:p + 1], scale=-1.0)
                    nc.vector.tensor_sub(
                        o2s[:, ks, 0:H],
                        sp[:, 0:kn, :], sn[:, 0:kn, :])

            def stage_t3(p):
                for g in range(2):
                    kg = bass.ds(g * 23, 23)
                    nc.scalar.dma_start(s3[:, kg, :],
                                        o2s[:, kg, :].rearrange("c k i -> c (k i)"),
                                        transpose=True)

            def stage_ri(p):
                nk2 = (K2 + K2CH - 1) // K2CH
                for ki in range(nk2):
                    k0 = ki * K2CH
                    kn = min(K2CH, K2 - k0)
                    rhs = s3[0:H, bass.ds(k0, kn), :].rearrange("p k c -> p c k")
                    psT = mm_psum.tile([H, BLK, K2CH], F32, tag="mm")
                    nc.tensor.matmul(psT[:, :, 0:kn],
                                     lhsT=rc_sb[:], rhs=rhs,
                                     start=True, stop=True)
                    nc.scalar.copy(
                        ttut[0:H, :, bass.ds(k0, kn)],
                        psT[:, :, 0:kn])
                    psU = mm_psum.tile([H, BLK, K2CH], F32, tag="mm")
                    nc.tensor.matmul(psU[:, :, 0:kn],
                                     lhsT=rs_sb[:], rhs=rhs,
                                     start=True, stop=True)
                    nc.vector.tensor_copy(
                        ttut[0:H, :, bass.ds(MU + k0, kn)],
                        psU[:, :, 0:kn])

            def stage_t4(p):
                for g, (h0, hn) in enumerate(((0, 64), (64, 32))):
                    nc.sync.dma_start(tu[:, :, bass.ds(h0, hn)],
                                      ttut[h0:h0 + hn, :, :]
                                      .rearrange("h c m -> h (c m)"),
                                      transpose=True)

            def stage_wi(p):
                for j in range(2):
                    for hg in range(H // (HCH * YGRP)):
                        h0g = hg * HCH * YGRP
                        ysb = y_pool.tile([90, HCH * YGRP, BLK], BF16, tag="y")
                        for si in range(YGRP):
                            h0 = h0g + si * HCH
                            hs = slice(h0, h0 + HCH)
                            ps = mm_psum.tile([90, HCH, BLK], F32, tag="mm")
                            nc.tensor.matmul(
                                ps[:], lhsT=cwi_sb[:, j, :],
                                rhs=tu[:, :, hs].rearrange("m c h -> m h c"),
                                start=True, stop=True)
                            dst = ysb[:, bass.ds(si * HCH, HCH), :]
                            if si % 2 == 0:
                                nc.vector.tensor_copy(dst, ps[:, :, :])
                            else:
                                nc.scalar.copy(dst, ps[:, :, :])
                        nc.sync.dma_start(
                            y_ext[p, j, :, h0g:h0g + HCH * YGRP, :], ysb[:])

            # software pipeline: inverse stages run one pass behind, giving
            # the PE independent work while each xbar transpose completes
            for it in range(NPASS + 1):
                if it < NPASS:
                    stage_mm1(it)
                    stage_t1(it)
                if it > 0:
                    stage_ri(it - 1)
                    stage_t4(it - 1)
                if it < NPASS:
                    stage_mm2(it)
                    stage_t2(it)
                if it > 0:
                    stage_wi(it - 1)
                if it < NPASS:
                    stage_mlp(it)
                    stage_t3(it)

    nc.compile()
    return nc


_CACHE = {}


def _get_graph():
    if "nc" not in _CACHE:
        _CACHE["nc"] = _build_graph()
    return _CACHE["nc"]


def kernel(x, w1, b1, w2, b2):
    x = np.ascontiguousarray(np.asarray(x, dtype=np.float32))
    w1 = np.asarray(w1, dtype=np.float32)
    b1 = np.asarray(b1, dtype=np.float32)
    w2 = np.asarray(w2, dtype=np.float32)
    b2 = np.asarray(b2, dtype=np.float32)

    wf8, rc, rs, cwi = _make_host_mats()
    nc = _get_graph()

    in_maps = []
    xcs = []
    for core in range(8):
        b = core // 2
        half = core % 2
        cs = half * CPC
        nb0 = half * NBPC
        xc = x[b, :, :, cs:cs + CPC]                      # [h, w, 384]
        xcs.append(xc)
        # [h, j, w', pass, c] -> [w', j, pass, h, c]
        x8 = np.ascontiguousarray(
            xc.reshape(H, 2, 90, NPASS, BLK).transpose(2, 1, 3, 0, 4)
        ).astype(E4)
        b2c = b2[0, nb0:nb0 + NBPC]                       # [NBPC, BLK]
        in_maps.append({
            "x8": x8,
            "wf8": wf8,
            "rc": rc,
            "rs": rs,
            "cwi": cwi,
            "w1b": w1[0, nb0:nb0 + NBPC].astype(BF),
            "w2b": w2[0, nb0:nb0 + NBPC].astype(BF),
            "b1b": np.ascontiguousarray(b1[0, nb0:nb0 + NBPC].T.astype(np.float32)),
            "b2m": np.ascontiguousarray((b2c - LAM).T.astype(np.float32)),
            "b2p": np.ascontiguousarray((-b2c - LAM).T.astype(np.float32)),
        })

    res = run_bass_kernel_spmd(nc, in_maps, core_ids=list(range(8)),
                               **_CACHE.get("run_kwargs", {}))
    _CACHE["last_result"] = res

    y = np.empty((B, H, W, C), np.float32)
    for core in range(8):
        b = core // 2
        cs = (core % 2) * CPC
        corr = res.results[core]["y"].astype(np.float32)  # [pass, j, w', h, c]
        # -> [h, j, w', pass, c] -> [h, w, 384]
        corr = corr.transpose(3, 1, 2, 0, 4).reshape(H, W, CPC)
        y[b, :, :, cs:cs + CPC] = xcs[core] + corr
    return y


if __name__ == "__main__":
    xs = np.random.randn(B, H, W, C).astype(np.float32)
    w1s = 0.02 * np.random.randn(2, NB, BLK, BLK).astype(np.float32)
    b1s = 0.02 * np.random.randn(2, NB, BLK).astype(np.float32)
    w2s = 0.02 * np.random.randn(2, NB, BLK, BLK).astype(np.float32)
    b2s = 0.02 * np.random.randn(2, NB, BLK).astype(np.float32)
    out = kernel(x=xs, w1=w1s, b1=b1s, w2=w2s, b2=b2s)
    print("ran, out shape", out.shape)


# revision 15
# speedup vs baseline: 1.8296x; 1.2770x over previous
"""AFNO2D Trainium2 kernel — 8 NeuronCores, no collectives.

Sharding: core = (b, c_half): b = core // 2, channels c_half*384 .. +384
(4 independent MLP blocks of 96 channels per core). Every stage is local.

The 2D Hartley transform (Re(FFT2) - Im(FFT2)) and its inverse are dense
matmuls against precomputed cos/sin matrices (H=90, W=180 fixed):
  MM1 (fp8 DoubleRow): P = PW @ x, M = MW @ x  (contract w=180 as [90,2])
  MM2: Xk = RC @ P + RS @ M                    (contract h)
  block-MLP per spectral point (2 layers, relu, softshrink)
  RI:  T = RC @ s, U = RS @ s                  (contract k1)
  WI:  corr = CWI @ [T;U]                      (contract m2-pad=128)

Partition-dim swaps between stages run as single batched DMA-crossbar
transposes (16x128 tiles), keeping the PE free for matmuls.

The residual add (y = x + corr) happens on the host: x is uploaded
pre-quantized to fp8 (only feeds the ~1%-magnitude spectral correction),
the correction is stored as bf16, and the f32 residual never loses
precision.
"""

import sys
import numpy as np

sys.path.insert(0, "/opt/trn_rl_repo")

import ml_dtypes  # noqa: E402
import concourse.bass as bass  # noqa: E402
import concourse.mybir as mybir  # noqa: E402
import concourse.tile as tile  # noqa: E402
from concourse import bacc  # noqa: E402
from concourse.bass_utils import run_bass_kernel_spmd  # noqa: E402

# problem constants (hardcoded per spec)
B, H, W, C = 4, 90, 180, 768
K2 = 46                 # kept width modes
NB, BLK = 8, 96
LAM = 0.01
CPC = 384               # channels per core
NBPC = 4                # MLP blocks per core
NPASS = 4               # c-passes of 96 channels (1 block) per core
HCH = 5                 # h-chunk -> N=480 for MM1/WI
CCH = 10                # c-chunk -> N=460 for MM2
K2CH = 5                # k2-chunk -> N=450 (MLP) / 480 (RI)
YGRP = 3                # WI h-chunks batched per store DMA
MU = 64                 # free-dim offset of the M/U halves (m2 hi part)

F32 = mybir.dt.float32
BF16 = mybir.dt.bfloat16
FP8 = mybir.dt.float8e4
BF = ml_dtypes.bfloat16
E4 = ml_dtypes.float8_e4m3


def _make_host_mats():
    w = np.arange(W)
    h = np.arange(H)
    k2 = np.arange(K2)
    beta = 2 * np.pi * np.outer(k2, w) / W            # [K2, W]
    PW = (np.cos(beta) + np.sin(beta)) / np.sqrt(W)
    MW = (np.cos(beta) - np.sin(beta)) / np.sqrt(W)
    # MM1 DoubleRow lhsT: [P/M, w', j, 48] (col 46,47 zero pad)
    wf8 = np.zeros((2, 90, 2, 48), np.float32)
    for j in range(2):
        wf8[0, :, j, :K2] = PW[:, j * 90:(j + 1) * 90].T
        wf8[1, :, j, :K2] = MW[:, j * 90:(j + 1) * 90].T
    alpha = 2 * np.pi * np.outer(h, h) / H            # [H, H] symmetric
    RC = np.cos(alpha) / np.sqrt(H)
    RS = np.sin(alpha) / np.sqrt(H)
    # WI lhsT [128, j, w']: rows k2 = cos, rows 64+k2 = -sin, rest zero
    cwi = np.zeros((128, 2, 90), np.float32)
    for j in range(2):
        ww = w[j * 90:(j + 1) * 90]
        bb = 2 * np.pi * np.outer(k2, ww) / W
        cwi[:K2, j, :] = np.cos(bb) / np.sqrt(W)
        cwi[MU:MU + K2, j, :] = -np.sin(bb) / np.sqrt(W)
    return (wf8.astype(E4), RC.astype(BF), RS.astype(BF), cwi.astype(BF))


def _build_graph():
    nc = bacc.Bacc("TRN2", target_bir_lowering=False, debug=False, num_devices=8)

    x8_ext = nc.declare_dram_parameter("x8", [90, 2, NPASS, H, BLK], FP8,
                                       isOutput=False)
    y_ext = nc.declare_dram_parameter("y", [NPASS, 2, 90, H, BLK], BF16,
                                      isOutput=True)
    wf8_ext = nc.declare_dram_parameter("wf8", [2, 90, 2, 48], FP8, isOutput=False)
    rc_ext = nc.declare_dram_parameter("rc", [H, H], BF16, isOutput=False)
    rs_ext = nc.declare_dram_parameter("rs", [H, H], BF16, isOutput=False)
    cwi_ext = nc.declare_dram_parameter("cwi", [128, 2, 90], BF16, isOutput=False)
    w1_ext = nc.declare_dram_parameter("w1b", [NBPC, BLK, BLK], BF16, isOutput=False)
    w2_ext = nc.declare_dram_parameter("w2b", [NBPC, BLK, BLK], BF16, isOutput=False)
    b1_ext = nc.declare_dram_parameter("b1b", [BLK, NBPC], F32, isOutput=False)
    # b2 folded with softshrink: b2m = b2 - lam (relu path 1), b2p = -b2 - lam
    b2m_ext = nc.declare_dram_parameter("b2m", [BLK, NBPC], F32, isOutput=False)
    b2p_ext = nc.declare_dram_parameter("b2p", [BLK, NBPC], F32, isOutput=False)

    DR = mybir.MatmulPerfMode.DoubleRow

    with tile.TileContext(nc) as tc:
        with (
            tc.tile_pool(name="consts", bufs=1) as consts,
            tc.tile_pool(name="x8", bufs=2) as x8_pool,
            tc.tile_pool(name="mid", bufs=1) as mid,
            tc.tile_pool(name="shr", bufs=2) as shr_pool,
            tc.tile_pool(name="ytile", bufs=3) as y_pool,
            tc.tile_pool(name="mmps", bufs=4, space="PSUM") as mm_psum,
        ):
            # ---- constants to SBUF ----
            wfP_sb = consts.tile([90, 2, 48], FP8)
            nc.sync.dma_start(wfP_sb[:], wf8_ext[0])
            wfM_sb = consts.tile([90, 2, 48], FP8)
            nc.sync.dma_start(wfM_sb[:], wf8_ext[1])
            rc_sb = consts.tile([H, H], BF16)
            nc.sync.dma_start(rc_sb[:], rc_ext[:])
            rs_sb = consts.tile([H, H], BF16)
            nc.sync.dma_start(rs_sb[:], rs_ext[:])
            cwi_sb = consts.tile([128, 2, 90], BF16)
            nc.sync.dma_start(cwi_sb[:], cwi_ext[:])
            w1_sb = consts.tile([BLK, NBPC, BLK], BF16)
            nc.sync.dma_start(w1_sb[:], w1_ext[:].rearrange("n i o -> i n o"))
            w2_sb = consts.tile([BLK, NBPC, BLK], BF16)
            nc.sync.dma_start(w2_sb[:], w2_ext[:].rearrange("n i o -> i n o"))
            b1_sb = consts.tile([BLK, NBPC], F32)
            nc.sync.dma_start(b1_sb[:], b1_ext[:])
            b2m_sb = consts.tile([BLK, NBPC], F32)
            nc.sync.dma_start(b2m_sb[:], b2m_ext[:])
            b2p_sb = consts.tile([BLK, NBPC], F32)
            nc.sync.dma_start(b2p_sb[:], b2p_ext[:])

            # ---- persistent intermediates (single-buffered) ----
            s1 = mid.tile([112, BLK, 128], BF16)      # [m2lay, c, h]
            s1t = mid.tile([128, BLK, 112], BF16)     # [h, c, m2lay]
            xk = mid.tile([96, K2, 128], BF16)        # [k1, k2, c]
            xkt = mid.tile([128, K2, 96], BF16)       # [c, k2, k1]
            o1 = mid.tile([BLK, K2, H], BF16)         # [c, k2, k1]
            o2s = mid.tile([BLK, K2, 128], BF16)      # [c, k2, k1pad]
            s3 = mid.tile([128, K2, BLK], BF16)       # [k1pad, k2, c]
            ttut = mid.tile([96, BLK, 128], BF16)     # [h, c, m2lay]
            tu = mid.tile([128, BLK, 96], BF16)       # [m2lay, c, h]
            # WI contracts all 128 tu rows against cwi (zero rows at the
            # pads); junk there must be 0.0, not NaN.
            nc.vector.memset(ttut[:, :, K2:MU], 0.0)
            nc.vector.memset(ttut[:, :, MU + K2:128], 0.0)
            # junk h-cols of o2s feed the k1 pad rows of s3 (never read)
            nc.vector.memset(o2s[:, :, H:128], 0.0)

            x8s = []
            for p in range(NPASS):
                t = x8_pool.tile([90, 2, H, BLK], FP8, tag="x8")
                x8s.append(t)
            nc.sync.dma_start(x8s[0][:], x8_ext[:, :, 0, :, :])

            def stage_mm1(p):
                x8t = x8s[p]
                if p + 1 < NPASS:
                    nc.sync.dma_start(x8s[p + 1][:], x8_ext[:, :, p + 1, :, :])
                for hi in range(H // HCH):
                    hs = slice(hi * HCH, (hi + 1) * HCH)
                    rhs = x8t[:, :, hs, :]
                    psP = mm_psum.tile([48, HCH, BLK], F32, tag="mm")
                    nc.tensor.matmul(psP[:], lhsT=wfP_sb[:], rhs=rhs,
                                     start=True, stop=True, perf_mode=DR)
                    psM = mm_psum.tile([48, HCH, BLK], F32, tag="mm")
                    nc.tensor.matmul(psM[:], lhsT=wfM_sb[:], rhs=rhs,
                                     start=True, stop=True, perf_mode=DR)
                    dstP = s1[0:K2, :, hs]
                    dstM = s1[MU:MU + K2, :, hs]
                    srcP = psP[0:K2, :, :].rearrange("p h c -> p c h")
                    srcM = psM[0:K2, :, :].rearrange("p h c -> p c h")
                    if hi % 2 == 0:
                        nc.scalar.copy(dstP, srcP)
                        nc.vector.tensor_copy(dstM, srcM)
                    else:
                        nc.vector.tensor_copy(dstP, srcP)
                        nc.scalar.copy(dstM, srcM)

            def stage_t1(p):
                for g in range(4):
                    cg = bass.ds(g * 24, 24)
                    nc.scalar.dma_start(s1t[:, cg, :],
                                        s1[:, cg, :].rearrange("p c h -> p (c h)"),
                                        transpose=True)

            def stage_mm2(p):
                ncch = (BLK + CCH - 1) // CCH
                for ci in range(ncch):
                    c0 = ci * CCH
                    cn = min(CCH, BLK - c0)
                    cs = bass.ds(c0, cn)
                    ps = mm_psum.tile([H, CCH, K2], F32, tag="mm")
                    nc.tensor.matmul(
                        ps[:, 0:cn, :], lhsT=rc_sb[:],
                        rhs=s1t[0:H, cs, 0:K2],
                        start=True, stop=False)
                    nc.tensor.matmul(
                        ps[:, 0:cn, :], lhsT=rs_sb[:],
                        rhs=s1t[0:H, cs, MU:MU + K2],
                        start=False, stop=True)
                    dst = xk[0:H, :, cs]
                    src_ = ps[:, 0:cn, :].rearrange("p c k -> p k c")
                    if ci % 2 == 0:
                        nc.scalar.copy(dst, src_)
                    else:
                        nc.vector.tensor_copy(dst, src_)

            def stage_t2(p):
                for g in range(2):
                    kg = bass.ds(g * 23, 23)
                    nc.sync.dma_start(xkt[:, kg, :],
                                      xk[:, kg, :].rearrange("p k c -> p (k c)"),
                                      transpose=True)

            def stage_mlp(p):
                nk2 = (K2 + K2CH - 1) // K2CH
                for ki in range(nk2):
                    k0 = ki * K2CH
                    kn = min(K2CH, K2 - k0)
                    ks = bass.ds(k0, kn)
                    ps = mm_psum.tile([BLK, K2CH, H], F32, tag="mm")
                    pss = ps[:, 0:kn, :]
                    nc.tensor.matmul(
                        pss, lhsT=w1_sb[:, p, :],
                        rhs=xkt[0:96, ks, 0:H],
                        start=True, stop=True)
                    nc.scalar.activation(
                        o1[:, ks, :],
                        pss, mybir.ActivationFunctionType.Relu,
                        bias=b1_sb[:, p:p + 1])
                for ki in range(nk2):
                    k0 = ki * K2CH
                    kn = min(K2CH, K2 - k0)
                    ks = bass.ds(k0, kn)
                    ps = mm_psum.tile([BLK, K2CH, H], F32, tag="mm")
                    pss = ps[:, 0:kn, :]
                    nc.tensor.matmul(
                        pss, lhsT=w2_sb[:, p, :],
                        rhs=o1[:, ks, :],
                        start=True, stop=True)
                    # softshrink(v + b2) = relu(v + b2 - lam) - relu(-v - b2 - lam)
                    sp = shr_pool.tile([BLK, K2CH, H], BF16, tag="shr_p")
                    sn = shr_pool.tile([BLK, K2CH, H], BF16, tag="shr_n")
                    nc.scalar.activation(
                        sp[:, 0:kn, :], pss, mybir.ActivationFunctionType.Relu,
                        bias=b2m_sb[:, p:p + 1], scale=1.0)
                    nc.scalar.activation(
                        sn[:, 0:kn, :], pss, mybir.ActivationFunctionType.Relu,
                        bias=b2p_sb[:, p:p + 1], scale=-1.0)
                    nc.vector.tensor_sub(
                        o2s[:, ks, 0:H],
                        sp[:, 0:kn, :], sn[:, 0:kn, :])

            def stage_t3(p):
                for g in range(2):
                    kg = bass.ds(g * 23, 23)
                    nc.scalar.dma_start(s3[:, kg, :],
                                        o2s[:, kg, :].rearrange("c k i -> c (k i)"),
                                        transpose=True)

            def stage_ri(p):
                nk2 = (K2 + K2CH - 1) // K2CH
                for ki in range(nk2):
                    k0 = ki * K2CH
                    kn = min(K2CH, K2 - k0)
                    rhs = s3[0:H, bass.ds(k0, kn), :]
                    psT = mm_psum.tile([H, K2CH, BLK], F32, tag="mm")
                    nc.tensor.matmul(psT[:, 0:kn, :],
                                     lhsT=rc_sb[:], rhs=rhs,
                                     start=True, stop=True)
                    nc.scalar.copy(
                        ttut[0:H, :, bass.ds(k0, kn)],
                        psT[:, 0:kn, :].rearrange("p k c -> p c k"))
                    psU = mm_psum.tile([H, K2CH, BLK], F32, tag="mm")
                    nc.tensor.matmul(psU[:, 0:kn, :],
                                     lhsT=rs_sb[:], rhs=rhs,
                                     start=True, stop=True)
                    nc.vector.tensor_copy(
                        ttut[0:H, :, bass.ds(MU + k0, kn)],
                        psU[:, 0:kn, :].rearrange("p k c -> p c k"))

            def stage_t4(p):
                for g, (h0, hn) in enumerate(((0, 64), (64, 32))):
                    nc.sync.dma_start(tu[:, :, bass.ds(h0, hn)],
                                      ttut[h0:h0 + hn, :, :]
                                      .rearrange("h c m -> h (c m)"),
                                      transpose=True)

            def stage_wi(p):
                for j in range(2):
                    for hg in range(H // (HCH * YGRP)):
                        h0g = hg * HCH * YGRP
                        ysb = y_pool.tile([90, HCH * YGRP, BLK], BF16, tag="y")
                        for si in range(YGRP):
                            h0 = h0g + si * HCH
                            hs = slice(h0, h0 + HCH)
                            ps = mm_psum.tile([90, BLK, HCH], F32, tag="mm")
                            nc.tensor.matmul(
                                ps[:], lhsT=cwi_sb[:, j, :],
                                rhs=tu[:, :, hs],
                                start=True, stop=True)
                            dst = ysb[:, bass.ds(si * HCH, HCH), :]
                            src_ = ps[:, :, :].rearrange("w c h -> w h c")
                            if si % 2 == 0:
                                nc.vector.tensor_copy(dst, src_)
                            else:
                                nc.scalar.copy(dst, src_)
                        nc.sync.dma_start(
                            y_ext[p, j, :, h0g:h0g + HCH * YGRP, :], ysb[:])

            # software pipeline: inverse stages run one pass behind, giving
            # the PE independent work while each xbar transpose completes
            for it in range(NPASS + 1):
                if it < NPASS:
                    stage_mm1(it)
                    stage_t1(it)
                if it > 0:
                    stage_ri(it - 1)
                    stage_t4(it - 1)
                if it < NPASS:
                    stage_mm2(it)
                    stage_t2(it)
                if it > 0:
                    stage_wi(it - 1)
                if it < NPASS:
                    stage_mlp(it)
                    stage_t3(it)

    nc.compile()
    return nc


_CACHE = {}


def _get_graph():
    if "nc" not in _CACHE:
        _CACHE["nc"] = _build_graph()
    return _CACHE["nc"]


def kernel(x, w1, b1, w2, b2):
    x = np.ascontiguousarray(np.asarray(x, dtype=np.float32))
    w1 = np.asarray(w1, dtype=np.float32)
    b1 = np.asarray(b1, dtype=np.float32)
    w2 = np.asarray(w2, dtype=np.float32)
    b2 = np.asarray(b2, dtype=np.float32)

    wf8, rc, rs, cwi = _make_host_mats()
    nc = _get_graph()

    in_maps = []
    xcs = []
    for core in range(8):
        b = core // 2
        half = core % 2
        cs = half * CPC
        nb0 = half * NBPC
        xc = x[b, :, :, cs:cs + CPC]                      # [h, w, 384]
        xcs.append(xc)
        # [h, j, w', pass, c] -> [w', j, pass, h, c]
        x8 = np.ascontiguousarray(
            xc.reshape(H, 2, 90, NPASS, BLK).transpose(2, 1, 3, 0, 4)
        ).astype(E4)
        b2c = b2[0, nb0:nb0 + NBPC]                       # [NBPC, BLK]
        in_maps.append({
            "x8": x8,
            "wf8": wf8,
            "rc": rc,
            "rs": rs,
            "cwi": cwi,
            "w1b": w1[0, nb0:nb0 + NBPC].astype(BF),
            "w2b": w2[0, nb0:nb0 + NBPC].astype(BF),
            "b1b": np.ascontiguousarray(b1[0, nb0:nb0 + NBPC].T.astype(np.float32)),
            "b2m": np.ascontiguousarray((b2c - LAM).T.astype(np.float32)),
            "b2p": np.ascontiguousarray((-b2c - LAM).T.astype(np.float32)),
        })

    res = run_bass_kernel_spmd(nc, in_maps, core_ids=list(range(8)),
                               **_CACHE.get("run_kwargs", {}))
    _CACHE["last_result"] = res

    y = np.empty((B, H, W, C), np.float32)
    for core in range(8):
        b = core // 2
        cs = (core % 2) * CPC
        corr = res.results[core]["y"].astype(np.float32)  # [pass, j, w', h, c]
        # -> [h, j, w', pass, c] -> [h, w, 384]
        corr = corr.transpose(3, 1, 2, 0, 4).reshape(H, W, CPC)
        y[b, :, :, cs:cs + CPC] = xcs[core] + corr
    return y


if __name__ == "__main__":
    xs = np.random.randn(B, H, W, C).astype(np.float32)
    w1s = 0.02 * np.random.randn(2, NB, BLK, BLK).astype(np.float32)
    b1s = 0.02 * np.random.randn(2, NB, BLK).astype(np.float32)
    w2s = 0.02 * np.random.randn(2, NB, BLK, BLK).astype(np.float32)
    b2s = 0.02 * np.random.randn(2, NB, BLK).astype(np.float32)
    out = kernel(x=xs, w1=w1s, b1=b1s, w2=w2s, b2=b2s)
    print("ran, out shape", out.shape)
